# revision 8
# baseline (speedup 1.0000x reference)
"""Trainium2 Bass kernel for ContextMatcher (retrieval_knn).

Accepts FULL inputs, shards rows across 8 NeuronCores internally,
returns the FULL 4-tuple (candidate_map, candidates, voronoi_mapping,
max_scores) matching the reference semantics.

Phase 1 (device): blocked cosine matmul [rows x 20000] in fp32 with
per-250-chunk top-8 extraction (max8/max_index), 7-round merge
(max8 + max_index + match_replace, exact f32 tie handling), then two
GPSIMD local_scatters to place the 50 winning global column indices in
rank order -- no per-partition gather needed.

Phase 2 (host): pool gather + sorted-unique padding (1024 small ints).

Phase 3 (device): voronoi matmul [rows x C] + max8/max_index for
first-occurrence argmax and max score.
"""

import numpy as np

import concourse.bass as bass  # noqa: F401  (bass must import before bacc)
import concourse.mybir as mybir
import concourse.tile as tile
from concourse import bacc, library_config
from concourse.bass_utils import run_bass_kernel_spmd

V = 20000
D = 256
NCORES = 8
RPC = V // NCORES          # 2500 rows per core
TILES = (RPC + 127) // 128  # 20 row-tiles (last padded)
RPAD = TILES * 128          # 2560
TOPK = 50
S_CH = 250                  # chunk width for stage-1 top-8 (containment verified)
NCHUNK = V // S_CH          # 80
CANDW = NCHUNK * 8          # 640 candidates per row
NROUND = 7                  # merge rounds -> 56 winners
NSEL = NROUND * 8           # 56
BW = 500                    # matmul N-block width (2 chunks, fits one PSUM bank)
NB = V // BW                # 40 N-blocks

_TRACE = False
_EXEC_NS = []
_P1_CACHE = {}
_P2_CACHE = {}

f32 = mybir.dt.float32
u16 = mybir.dt.uint16
i16 = mybir.dt.int16


def _build_phase1():
    nc = bacc.Bacc("TRN2", target_bir_lowering=False, debug=False,
                   num_devices=NCORES)
    embT = nc.dram_tensor("embT", [2, 128, V], f32, kind="ExternalInput")
    myT = nc.dram_tensor("myT", [2, 128, RPAD], f32, kind="ExternalInput")
    offs_in = nc.dram_tensor("offs", [128, CANDW], u16, kind="ExternalInput")
    ranks_in = nc.dram_tensor("ranks", [128, NSEL], u16, kind="ExternalInput")
    out_cmap = nc.dram_tensor("cmap", [TILES, 128, 64], u16,
                              kind="ExternalOutput")

    with tile.TileContext(nc) as tc:
        with (
            tc.tile_pool(name="persist", bufs=1) as persist,
            tc.tile_pool(name="lhs", bufs=3) as lhsp,
            tc.tile_pool(name="psum", bufs=8, space="PSUM") as psp,
            tc.tile_pool(name="cand", bufs=2) as candp,
            tc.tile_pool(name="small", bufs=2) as smallp,
        ):
            nc.gpsimd.load_library(library_config.local_scatter)
            emb_a = persist.tile([128, V], f32, tag="emba")
            emb_b = persist.tile([128, V], f32, tag="embb")
            nc.sync.dma_start(emb_a[:], embT[0])
            nc.sync.dma_start(emb_b[:], embT[1])
            offs = persist.tile([128, CANDW], u16, tag="offs")
            nc.sync.dma_start(offs[:], offs_in[:])
            ranks = persist.tile([128, NSEL], u16, tag="ranks")
            nc.sync.dma_start(ranks[:], ranks_in[:])

            for t in range(TILES):
                lhs_a = lhsp.tile([128, 128], f32, tag="lhsa")
                lhs_b = lhsp.tile([128, 128], f32, tag="lhsb")
                nc.sync.dma_start(lhs_a[:], myT[0, :, t * 128:(t + 1) * 128])
                nc.sync.dma_start(lhs_b[:], myT[1, :, t * 128:(t + 1) * 128])

                cvals = candp.tile([128, CANDW], f32, tag="cvals")
                lidx = candp.tile([128, CANDW], u16, tag="lidx")

                for b in range(NB):
                    ps = psp.tile([128, BW], f32, tag="ps")
                    nc.tensor.matmul(ps[:], lhs_a[:],
                                     emb_a[:, b * BW:(b + 1) * BW],
                                     start=True, stop=False)
                    nc.tensor.matmul(ps[:], lhs_b[:],
                                     emb_b[:, b * BW:(b + 1) * BW],
                                     start=False, stop=True)
                    for j in range(2):
                        c = 2 * b + j
                        nc.vector.max(
                            out=cvals[:, c * 8:(c + 1) * 8],
                            in_=ps[:, j * S_CH:(j + 1) * S_CH])
                        nc.vector.max_index(
                            out=lidx[:, c * 8:(c + 1) * 8],
                            in_max=cvals[:, c * 8:(c + 1) * 8],
                            in_values=ps[:, j * S_CH:(j + 1) * S_CH])

                glob = candp.tile([128, CANDW], u16, tag="glob")
                nc.vector.tensor_add(glob[:], lidx[:], offs[:])

                pos = smallp.tile([128, NSEL], u16, tag="pos")
                w8 = smallp.tile([128, 8], f32, tag="w8")
                for r in range(NROUND):
                    nc.vector.max(out=w8[:], in_=cvals[:])
                    nc.vector.max_index(out=pos[:, r * 8:(r + 1) * 8],
                                        in_max=w8[:], in_values=cvals[:])
                    if r < NROUND - 1:
                        nc.vector.match_replace(out=cvals[:], in_to_replace=w8[:],
                                                in_values=cvals[:],
                                                imm_value=-1e30)

                pos16 = smallp.tile([128, NSEL], i16, tag="pos16")
                nc.vector.tensor_copy(pos16[:], pos[:])
                dst1 = candp.tile([128, CANDW], u16, tag="dst1")
                nc.gpsimd.local_scatter(dst1[:], ranks[:], pos16[:],
                                        128, CANDW, NSEL)
                d1i = candp.tile([128, CANDW], i16, tag="d1i")
                nc.vector.tensor_scalar(d1i[:], dst1[:].bitcast(i16), 1.0,
                                        scalar2=None,
                                        op0=mybir.AluOpType.subtract)
                dst2 = smallp.tile([128, 64], u16, tag="dst2")
                nc.gpsimd.local_scatter(dst2[:], glob[:], d1i[:],
                                        128, 64, CANDW)
                nc.sync.dma_start(out_cmap[t], dst2[:])

    nc.compile()
    return nc


def _build_phase2(C):
    nc = bacc.Bacc("TRN2", target_bir_lowering=False, debug=False,
                   num_devices=NCORES)
    myT = nc.dram_tensor("myT", [2, 128, RPAD], f32, kind="ExternalInput")
    candT = nc.dram_tensor("candT", [2, 128, C], f32, kind="ExternalInput")
    out_vor = nc.dram_tensor("vor", [TILES, 128, 1], u16,
                             kind="ExternalOutput")
    out_ms = nc.dram_tensor("ms", [TILES, 128, 1], f32,
                            kind="ExternalOutput")
    NB2 = (C + 511) // 512

    with tile.TileContext(nc) as tc:
        with (
            tc.tile_pool(name="persist", bufs=1) as persist,
            tc.tile_pool(name="lhs", bufs=4) as lhsp,
            tc.tile_pool(name="psum", bufs=4, space="PSUM") as psp,
            tc.tile_pool(name="small", bufs=4) as smallp,
        ):
            cnd_a = persist.tile([128, C], f32, tag="cnda")
            cnd_b = persist.tile([128, C], f32, tag="cndb")
            nc.sync.dma_start(cnd_a[:], candT[0])
            nc.sync.dma_start(cnd_b[:], candT[1])

            for t in range(TILES):
                lhs_a = lhsp.tile([128, 128], f32, tag="lhsa")
                lhs_b = lhsp.tile([128, 128], f32, tag="lhsb")
                nc.sync.dma_start(lhs_a[:], myT[0, :, t * 128:(t + 1) * 128])
                nc.sync.dma_start(lhs_b[:], myT[1, :, t * 128:(t + 1) * 128])

                ps = psp.tile([128, NB2 * 512], f32, tag="ps")
                for b in range(NB2):
                    n0 = b * 512
                    n1 = min(C, n0 + 512)
                    nc.tensor.matmul(ps[:, n0:n1], lhs_a[:], cnd_a[:, n0:n1],
                                     start=True, stop=False)
                    nc.tensor.matmul(ps[:, n0:n1], lhs_b[:], cnd_b[:, n0:n1],
                                     start=False, stop=True)

                mx8 = smallp.tile([128, 8], f32, tag="mx8")
                ix8 = smallp.tile([128, 8], u16, tag="ix8")
                nc.vector.max(out=mx8[:], in_=ps[:, 0:C])
                nc.vector.max_index(out=ix8[:], in_max=mx8[:],
                                    in_values=ps[:, 0:C])
                nc.sync.dma_start(out_vor[t], ix8[:, 0:1])
                nc.sync.dma_start(out_ms[t], mx8[:, 0:1])

    nc.compile()
    return nc


def _run(nc, in_maps):
    global _EXEC_NS
    res = run_bass_kernel_spmd(nc, in_maps, list(range(NCORES)),
                               trace=_TRACE)
    if _TRACE:
        _EXEC_NS.append(res.exec_time_ns)
    return res.results


def kernel(embeddings, x, K):
    emb = np.ascontiguousarray(np.asarray(embeddings), dtype=np.float32)
    x = np.asarray(x).astype(np.int64)
    K = int(K)
    B = x.shape[0]

    nrm = np.sqrt((emb * emb).sum(axis=1, keepdims=True, dtype=np.float32))
    embn = emb / np.maximum(nrm, np.float32(1e-12))

    embT_arr = np.ascontiguousarray(embn.T).reshape(2, 128, V)
    offs_arr = np.broadcast_to(
        ((np.arange(CANDW) // 8) * S_CH).astype(np.uint16), (128, CANDW))
    offs_arr = np.ascontiguousarray(offs_arr)
    ranks_arr = np.ascontiguousarray(np.broadcast_to(
        np.arange(1, NSEL + 1, dtype=np.uint16), (128, NSEL)))

    my_t = []
    for c in range(NCORES):
        m = np.zeros((D, RPAD), dtype=np.float32)
        m[:, :RPC] = embn[c * RPC:(c + 1) * RPC].T
        my_t.append(np.ascontiguousarray(m).reshape(2, 128, RPAD))

    if "p1" not in _P1_CACHE:
        _P1_CACHE["p1"] = _build_phase1()
    nc1 = _P1_CACHE["p1"]
    in_maps1 = [{"embT": embT_arr, "myT": my_t[c], "offs": offs_arr,
                 "ranks": ranks_arr} for c in range(NCORES)]
    res1 = _run(nc1, in_maps1)

    cmap_full = np.zeros((V, 64), dtype=np.uint16)
    for c in range(NCORES):
        rows = res1[c]["cmap"].reshape(RPAD, 64)[:RPC]
        cmap_full[c * RPC:(c + 1) * RPC] = rows
    candidate_map = cmap_full[:, :TOPK].astype(np.int32)

    # Phase 2 on host: pool gather + sorted unique with zero padding.
    C = B * K
    pool = candidate_map[x, :K].reshape(-1)
    u = np.unique(pool)
    candidates = np.zeros(C, dtype=np.int32)
    candidates[:min(len(u), C)] = u[:C]

    candT_arr = np.ascontiguousarray(embn[candidates].T).reshape(2, 128, C)
    key = ("p2", C)
    if key not in _P2_CACHE:
        _P2_CACHE[key] = _build_phase2(C)
    nc2 = _P2_CACHE[key]
    in_maps2 = [{"myT": my_t[c], "candT": candT_arr} for c in range(NCORES)]
    res2 = _run(nc2, in_maps2)

    voronoi = np.zeros(V, dtype=np.int32)
    max_scores = np.zeros(V, dtype=np.float32)
    for c in range(NCORES):
        voronoi[c * RPC:(c + 1) * RPC] = \
            res2[c]["vor"].reshape(RPAD)[:RPC].astype(np.int32)
        max_scores[c * RPC:(c + 1) * RPC] = res2[c]["ms"].reshape(RPAD)[:RPC]

    return candidate_map, candidates, voronoi, max_scores


# revision 10
# speedup vs baseline: 1.1215x; 1.1215x over previous
"""Trainium2 Bass kernel for ContextMatcher (retrieval_knn).

Accepts FULL inputs, shards rows across 8 NeuronCores internally,
returns the FULL 4-tuple (candidate_map, candidates, voronoi_mapping,
max_scores) matching the reference semantics.

Phase 1 (device): blocked cosine matmul [rows x 20000] in fp32 with
per-250-chunk top-8 extraction (max8/max_index), 7-round merge
(max8 + max_index + match_replace, exact f32 tie handling), then two
GPSIMD local_scatters to place the 50 winning global column indices in
rank order -- no per-partition gather needed.

Phase 2 (host): pool gather + sorted-unique padding (1024 small ints).

Phase 3 (device): voronoi matmul [rows x C] + max8/max_index for
first-occurrence argmax and max score.
"""

import numpy as np

import concourse.bass as bass  # noqa: F401  (bass must import before bacc)
import concourse.mybir as mybir
import concourse.tile as tile
from concourse import bacc, library_config
from concourse.bass_utils import run_bass_kernel_spmd

V = 20000
D = 256
NCORES = 8
RPC = V // NCORES          # 2500 rows per core
TILES = (RPC + 127) // 128  # 20 row-tiles (last padded)
RPAD = TILES * 128          # 2560
TOPK = 50
S_CH = 250                  # chunk width for stage-1 top-8 (containment verified)
NCHUNK = V // S_CH          # 80
CANDW = NCHUNK * 8          # 640 candidates per row
NROUND = 7                  # merge rounds -> 56 winners
NSEL = NROUND * 8           # 56
BW = 500                    # matmul N-block width (2 chunks, fits one PSUM bank)
NB = V // BW                # 40 N-blocks

_TRACE = False
_EXEC_NS = []
_P1_CACHE = {}
_P2_CACHE = {}

f32 = mybir.dt.float32
u16 = mybir.dt.uint16
i16 = mybir.dt.int16


def _build_phase1():
    nc = bacc.Bacc("TRN2", target_bir_lowering=False, debug=False,
                   num_devices=NCORES)
    embT = nc.dram_tensor("embT", [2, 128, V], f32, kind="ExternalInput")
    myT = nc.dram_tensor("myT", [2, 128, RPAD], f32, kind="ExternalInput")
    offs_in = nc.dram_tensor("offs", [128, CANDW], u16, kind="ExternalInput")
    ranks_in = nc.dram_tensor("ranks", [128, NSEL], u16, kind="ExternalInput")
    out_cmap = nc.dram_tensor("cmap", [TILES, 128, 64], u16,
                              kind="ExternalOutput")

    with tile.TileContext(nc) as tc:
        with (
            tc.tile_pool(name="persist", bufs=1) as persist,
            tc.tile_pool(name="lhs", bufs=3) as lhsp,
            tc.tile_pool(name="psum", bufs=8, space="PSUM") as psp,
            tc.tile_pool(name="slab", bufs=6) as slabp,
            tc.tile_pool(name="cand", bufs=2) as candp,
            tc.tile_pool(name="small", bufs=2) as smallp,
        ):
            nc.gpsimd.load_library(library_config.local_scatter)
            emb_a = persist.tile([128, V], f32, tag="emba")
            emb_b = persist.tile([128, V], f32, tag="embb")
            nc.sync.dma_start(emb_a[:], embT[0])
            nc.sync.dma_start(emb_b[:], embT[1])
            offs = persist.tile([128, CANDW], u16, tag="offs")
            nc.sync.dma_start(offs[:], offs_in[:])
            ranks = persist.tile([128, NSEL], u16, tag="ranks")
            nc.sync.dma_start(ranks[:], ranks_in[:])

            for t in range(TILES):
                lhs_a = lhsp.tile([128, 128], f32, tag="lhsa")
                lhs_b = lhsp.tile([128, 128], f32, tag="lhsb")
                nc.sync.dma_start(lhs_a[:], myT[0, :, t * 128:(t + 1) * 128])
                nc.sync.dma_start(lhs_b[:], myT[1, :, t * 128:(t + 1) * 128])

                cvals = candp.tile([128, CANDW], f32, tag="cvals")
                lidx = candp.tile([128, CANDW], u16, tag="lidx")

                for b in range(NB):
                    ps = psp.tile([128, BW], f32, tag="ps")
                    nc.tensor.matmul(ps[:], lhs_a[:],
                                     emb_a[:, b * BW:(b + 1) * BW],
                                     start=True, stop=False)
                    nc.tensor.matmul(ps[:], lhs_b[:],
                                     emb_b[:, b * BW:(b + 1) * BW],
                                     start=False, stop=True)
                    sl = slabp.tile([128, BW], f32, tag="sl")
                    nc.scalar.copy(sl[:], ps[:])
                    for j in range(2):
                        c = 2 * b + j
                        nc.vector.max(
                            out=cvals[:, c * 8:(c + 1) * 8],
                            in_=sl[:, j * S_CH:(j + 1) * S_CH])
                        nc.vector.max_index(
                            out=lidx[:, c * 8:(c + 1) * 8],
                            in_max=cvals[:, c * 8:(c + 1) * 8],
                            in_values=sl[:, j * S_CH:(j + 1) * S_CH])

                glob = candp.tile([128, CANDW], u16, tag="glob")
                nc.vector.tensor_add(glob[:], lidx[:], offs[:])

                pos = smallp.tile([128, NSEL], u16, tag="pos")
                w8 = smallp.tile([128, 8], f32, tag="w8")
                for r in range(NROUND):
                    nc.vector.max(out=w8[:], in_=cvals[:])
                    nc.vector.max_index(out=pos[:, r * 8:(r + 1) * 8],
                                        in_max=w8[:], in_values=cvals[:])
                    if r < NROUND - 1:
                        nc.vector.match_replace(out=cvals[:], in_to_replace=w8[:],
                                                in_values=cvals[:],
                                                imm_value=-1e30)

                pos16 = smallp.tile([128, NSEL], i16, tag="pos16")
                nc.vector.tensor_copy(pos16[:], pos[:])
                dst1 = candp.tile([128, CANDW], u16, tag="dst1")
                nc.gpsimd.local_scatter(dst1[:], ranks[:], pos16[:],
                                        128, CANDW, NSEL)
                d1i = candp.tile([128, CANDW], i16, tag="d1i")
                nc.vector.tensor_scalar(d1i[:], dst1[:].bitcast(i16), 1.0,
                                        scalar2=None,
                                        op0=mybir.AluOpType.subtract)
                dst2 = smallp.tile([128, 64], u16, tag="dst2")
                nc.gpsimd.local_scatter(dst2[:], glob[:], d1i[:],
                                        128, 64, CANDW)
                nc.sync.dma_start(out_cmap[t], dst2[:])

    nc.compile()
    return nc


def _build_phase2(C):
    nc = bacc.Bacc("TRN2", target_bir_lowering=False, debug=False,
                   num_devices=NCORES)
    myT = nc.dram_tensor("myT", [2, 128, RPAD], f32, kind="ExternalInput")
    candT = nc.dram_tensor("candT", [2, 128, C], f32, kind="ExternalInput")
    out_vor = nc.dram_tensor("vor", [TILES, 128, 1], u16,
                             kind="ExternalOutput")
    out_ms = nc.dram_tensor("ms", [TILES, 128, 1], f32,
                            kind="ExternalOutput")
    NB2 = (C + 511) // 512

    with tile.TileContext(nc) as tc:
        with (
            tc.tile_pool(name="persist", bufs=1) as persist,
            tc.tile_pool(name="lhs", bufs=4) as lhsp,
            tc.tile_pool(name="psum", bufs=4, space="PSUM") as psp,
            tc.tile_pool(name="small", bufs=4) as smallp,
        ):
            cnd_a = persist.tile([128, C], f32, tag="cnda")
            cnd_b = persist.tile([128, C], f32, tag="cndb")
            nc.sync.dma_start(cnd_a[:], candT[0])
            nc.sync.dma_start(cnd_b[:], candT[1])

            for t in range(TILES):
                lhs_a = lhsp.tile([128, 128], f32, tag="lhsa")
                lhs_b = lhsp.tile([128, 128], f32, tag="lhsb")
                nc.sync.dma_start(lhs_a[:], myT[0, :, t * 128:(t + 1) * 128])
                nc.sync.dma_start(lhs_b[:], myT[1, :, t * 128:(t + 1) * 128])

                ps = psp.tile([128, NB2 * 512], f32, tag="ps")
                for b in range(NB2):
                    n0 = b * 512
                    n1 = min(C, n0 + 512)
                    nc.tensor.matmul(ps[:, n0:n1], lhs_a[:], cnd_a[:, n0:n1],
                                     start=True, stop=False)
                    nc.tensor.matmul(ps[:, n0:n1], lhs_b[:], cnd_b[:, n0:n1],
                                     start=False, stop=True)

                mx8 = smallp.tile([128, 8], f32, tag="mx8")
                ix8 = smallp.tile([128, 8], u16, tag="ix8")
                nc.vector.max(out=mx8[:], in_=ps[:, 0:C])
                nc.vector.max_index(out=ix8[:], in_max=mx8[:],
                                    in_values=ps[:, 0:C])
                nc.sync.dma_start(out_vor[t], ix8[:, 0:1])
                nc.sync.dma_start(out_ms[t], mx8[:, 0:1])

    nc.compile()
    return nc


def _run(nc, in_maps):
    global _EXEC_NS
    res = run_bass_kernel_spmd(nc, in_maps, list(range(NCORES)),
                               trace=_TRACE)
    if _TRACE:
        _EXEC_NS.append(res.exec_time_ns)
    return res.results


def kernel(embeddings, x, K):
    emb = np.ascontiguousarray(np.asarray(embeddings), dtype=np.float32)
    x = np.asarray(x).astype(np.int64)
    K = int(K)
    B = x.shape[0]

    nrm = np.sqrt((emb * emb).sum(axis=1, keepdims=True, dtype=np.float32))
    embn = emb / np.maximum(nrm, np.float32(1e-12))

    embT_arr = np.ascontiguousarray(embn.T).reshape(2, 128, V)
    offs_arr = np.broadcast_to(
        ((np.arange(CANDW) // 8) * S_CH).astype(np.uint16), (128, CANDW))
    offs_arr = np.ascontiguousarray(offs_arr)
    ranks_arr = np.ascontiguousarray(np.broadcast_to(
        np.arange(1, NSEL + 1, dtype=np.uint16), (128, NSEL)))

    my_t = []
    for c in range(NCORES):
        m = np.zeros((D, RPAD), dtype=np.float32)
        m[:, :RPC] = embn[c * RPC:(c + 1) * RPC].T
        my_t.append(np.ascontiguousarray(m).reshape(2, 128, RPAD))

    if "p1" not in _P1_CACHE:
        _P1_CACHE["p1"] = _build_phase1()
    nc1 = _P1_CACHE["p1"]
    in_maps1 = [{"embT": embT_arr, "myT": my_t[c], "offs": offs_arr,
                 "ranks": ranks_arr} for c in range(NCORES)]
    res1 = _run(nc1, in_maps1)

    cmap_full = np.zeros((V, 64), dtype=np.uint16)
    for c in range(NCORES):
        rows = res1[c]["cmap"].reshape(RPAD, 64)[:RPC]
        cmap_full[c * RPC:(c + 1) * RPC] = rows
    candidate_map = cmap_full[:, :TOPK].astype(np.int32)

    # Phase 2 on host: pool gather + sorted unique with zero padding.
    C = B * K
    pool = candidate_map[x, :K].reshape(-1)
    u = np.unique(pool)
    candidates = np.zeros(C, dtype=np.int32)
    candidates[:min(len(u), C)] = u[:C]

    candT_arr = np.ascontiguousarray(embn[candidates].T).reshape(2, 128, C)
    key = ("p2", C)
    if key not in _P2_CACHE:
        _P2_CACHE[key] = _build_phase2(C)
    nc2 = _P2_CACHE[key]
    in_maps2 = [{"myT": my_t[c], "candT": candT_arr} for c in range(NCORES)]
    res2 = _run(nc2, in_maps2)

    voronoi = np.zeros(V, dtype=np.int32)
    max_scores = np.zeros(V, dtype=np.float32)
    for c in range(NCORES):
        voronoi[c * RPC:(c + 1) * RPC] = \
            res2[c]["vor"].reshape(RPAD)[:RPC].astype(np.int32)
        max_scores[c * RPC:(c + 1) * RPC] = res2[c]["ms"].reshape(RPAD)[:RPC]

    return candidate_map, candidates, voronoi, max_scores
